# revision 2
# baseline (speedup 1.0000x reference)
"""Fused 2-layer KAN for Trainium2, data-parallel across 8 NeuronCores.

Math: with G=3 grid points the spline basis is piecewise-linear in x, so each
KAN layer collapses to a small dense matmul over 3 cheap feature maps:

    out = bias + silu(x) @ Wb + u @ P1 + C @ (P2 - P1)
      u = clip(x, -1, 1),  C = max(u, 0)
      Wb = imp*bw;  T = imp*sw*cp;  P1 = T@(bv1-bv0);  P2 = T@(bv2-bv1)
      bias_j = sum_i T[i,j,:] @ bv1

All K=5 spline control points fold into P1/P2/bias on the host (O(I*J*K) work).

Device layout (per 1024-row macro-tile, per core):
  partition p of the input tile holds rows {4p, 4p+1, 4p+2, 4p+3} of a
  512-row half-macro -> every DMA descriptor moves 1 KiB contiguous HBM
  (>=512B line-rate threshold), in and out.
  DMA in (SWDGE f32->bf16 cast) -> PE transpose to feature-major
  -> {silu(ACT), clip(DVE), relu-clip(DVE)} -> L1 row-tiled matmul pairs
  into one 2-bank PSUM tile -> L2 maps (ACT/DVE bank-crossed, c2 on GpSimd)
  -> 24 L2 block matmuls (bias via K=1 ones-matmul PSUM init)
  -> PSUM->SBUF copy (alternating ACT/DVE per macro) -> DMA out (HWDGE).
"""

import os
import sys
from contextlib import ExitStack

import numpy as np
import ml_dtypes

for _p in ("/opt/trn_rl_repo",):
    if _p not in sys.path and os.path.isdir(_p):
        sys.path.insert(0, _p)

import concourse.bass as bass
import concourse.tile as tile
from concourse import bacc, mybir
from concourse.bass_utils import run_bass_kernel_spmd
from concourse.masks import make_identity

F32 = mybir.dt.float32
BF16 = mybir.dt.bfloat16
BF = ml_dtypes.bfloat16

N_CORES = 8
D0, D1, D2 = 64, 128, 64
K, DEG, G, LO, HI = 5, 3, 3, -1.0, 1.0
MACRO = 1024  # batch rows per device macro-iteration

_nc_cache = {}


def _basis_table():
    knots = np.linspace(LO - DEG * 0.1, HI + DEG * 0.1, K + DEG + 1)
    grid = np.linspace(LO, HI, G)
    bv = np.zeros((G, K), dtype=np.float32)
    for i in range(K):
        center = (knots[i + DEG // 2] + knots[i + DEG // 2 + 1]) / 2.0
        width = (knots[i + DEG + 1] - knots[i]) / 2.0
        bv[:, i] = np.exp(-(((grid - center) / width) ** 2))
    bv = bv / (bv.sum(axis=1, keepdims=True) + 1e-6)
    return bv


def _prep_consts(cp0, bw0, sw0, imp0, cp1, bw1, sw1, imp1):
    f8 = np.float64
    bv = _basis_table().astype(f8)
    d1, d2 = bv[1] - bv[0], bv[2] - bv[1]

    def fold(cp, bw, sw, imp):
        T = imp.astype(f8)[:, :, None] * sw.astype(f8)[:, :, None] * cp.astype(f8)
        Wb = imp.astype(f8) * bw.astype(f8)
        return Wb, T @ d1, T @ d2, (T @ bv[1]).sum(axis=0)

    Wb0, P10, P20, b1 = fold(cp0, bw0, sw0, imp0)
    Wb1, P11, P21, b2 = fold(cp1, bw1, sw1, imp1)
    bias2_eff = b2 + b1 @ P21

    w1 = np.stack([Wb0, P10, P20 - P10], axis=0)  # [3, 64, 128] lhsT chunks
    w1 = np.concatenate([w1, w1], axis=1)  # duplicate rows for partitions 64-127
    w1 = np.ascontiguousarray(w1.transpose(1, 0, 2)).reshape(128, 384)
    w2 = np.stack([Wb1, P11, P21 - P11], axis=0)  # [3, 128, 64] rhs chunks
    w2 = np.ascontiguousarray(w2.transpose(1, 0, 2)).reshape(128, 192)

    return {
        "wpk": np.concatenate([w1, w2], axis=1).astype(BF),  # [128, 576]
        "spk": np.stack(
            [b1, -1.0 - b1, 1.0 - b1, -b1], axis=1
        ).astype(np.float32),  # [128, 4] = b1|s1|s2|nb1
        "b2row": np.tile(bias2_eff, 8).astype(BF).reshape(1, 512),
    }


def _build(rows):
    assert rows % MACRO == 0
    nc = bacc.Bacc(
        "TRN2",
        target_bir_lowering=False,
        debug=False,
        enable_asserts=False,
        num_devices=N_CORES,
    )
    xd = nc.dram_tensor("x", [rows, D0], F32, kind="ExternalInput")
    wpkd = nc.dram_tensor("wpk", [128, 576], BF16, kind="ExternalInput")
    spkd = nc.dram_tensor("spk", [128, 4], F32, kind="ExternalInput")
    b2d = nc.dram_tensor("b2row", [1, 512], BF16, kind="ExternalInput")
    outd = nc.dram_tensor("out", [rows, D2], F32, kind="ExternalOutput")

    n_macro = rows // MACRO
    MAX, MIN = mybir.AluOpType.max, mybir.AluOpType.min
    SILU = mybir.ActivationFunctionType.Silu

    with tile.TileContext(nc) as tc, ExitStack() as ctx:
        consts = ctx.enter_context(tc.tile_pool(name="consts", bufs=1))
        xin = ctx.enter_context(tc.tile_pool(name="xin", bufs=4))
        f1 = ctx.enter_context(tc.tile_pool(name="f1", bufs=2))
        f2 = ctx.enter_context(tc.tile_pool(name="f2", bufs=2))
        osb = ctx.enter_context(tc.tile_pool(name="osb", bufs=3))
        ps_x = ctx.enter_context(tc.tile_pool(name="ps_x", bufs=2, space="PSUM"))
        ps_h = ctx.enter_context(tc.tile_pool(name="ps_h", bufs=2, space="PSUM"))
        ps_o = ctx.enter_context(tc.tile_pool(name="ps_o", bufs=2, space="PSUM"))

        ident = consts.tile([128, 128], BF16)
        make_identity(nc, ident)
        ones = consts.tile([1, 128], BF16)
        nc.vector.memset(ones, 1.0)
        wpk = consts.tile([128, 576], BF16)
        nc.sync.dma_start(wpk, wpkd.ap())
        spk = consts.tile([128, 4], F32)
        nc.sync.dma_start(spk, spkd.ap())
        b2r = consts.tile([1, 512], BF16)
        nc.sync.dma_start(b2r, b2d.ap())
        b1, s1, s2, nb1 = (spk[:, i : i + 1] for i in range(4))
        w1c = [wpk[:, c * 128 : (c + 1) * 128] for c in range(3)]
        w2c = [wpk[:, 384 + c * 64 : 384 + (c + 1) * 64] for c in range(3)]

        # PE pre-warm: dummy matmuls spanning ~4us while the first DMAs land,
        # so the HAM clock gate opens (1.2 -> 2.4 GHz) before real work.
        warm = ps_o.tile([128, 2, 4, 64], F32, tag="po")
        for _ in range(48):
            nc.tensor.matmul(warm[:, 0, 0, 0:2], ident, ident[:, 0:2], start=True, stop=True)

        for m in range(n_macro):
            base = m * MACRO
            # xt[p, b, t, j, f] = x[base + 512b + 4p + 2t + j, f], bf16 cast.
            # Per partition: 2 chunks (b) of 256 contiguous f32 = 1KiB HBM.
            xt = xin.tile([128, 2, 2, 2, 64], BF16, tag="xt")
            src = bass.AP(
                xd,
                base * 64,
                [[4 * 64, 128], [512 * 64, 2], [2 * 64, 2], [64, 2], [1, 64]],
            )
            nc.gpsimd.dma_start(xt, src)

            # transpose: px[j*64+f, b, t, p] = x[base + 512b + 4p + 2t + j, f]
            # partitions 0-63 = feats of even rows (j=0), 64-127 = odd (j=1)
            px = ps_x.tile([128, 2, 2, 128], BF16, tag="px")
            for b in range(2):
                for t in range(2):
                    nc.tensor.transpose(px[:, b, t], xt[:, b, t], ident)

            # u1 first: then c1 (DVE, from SBUF) overlaps sl1 (ACT, from PSUM)
            u1 = f1.tile([128, 2, 2, 128], BF16, tag="u1")
            nc.vector.tensor_scalar(u1, px, -1.0, 1.0, op0=MAX, op1=MIN)
            sl1 = f1.tile([128, 2, 2, 128], BF16, tag="sl1")
            nc.scalar.activation(sl1, px, SILU)
            c1 = f1.tile([128, 2, 2, 128], BF16, tag="c1")
            nc.vector.tensor_scalar_max(c1, u1, 0.0)

            # L1: two concurrent 64-contraction row-tiled streams into one
            # 2-bank PSUM tile: h[d1, j, b, t, p]; j=0 -> bank A, j=1 -> bank B
            h = ps_h.tile([128, 2, 2, 2, 128], F32, tag="h")
            # chunk order = feature readiness order (u1 -> sl1 -> c1)
            for i, (c, ft) in enumerate([(1, u1), (0, sl1), (2, c1)]):
                nc.tensor.matmul(
                    h[:, 0], w1c[c][0:64], ft[0:64], start=(i == 0), stop=(i == 2)
                )
                nc.tensor.matmul(
                    h[:, 1], w1c[c][64:128], ft[64:128], start=(i == 0), stop=(i == 2)
                )

            # L2 bias init via K=1 ones-matmul (sets has_written on the whole
            # bank so the 24 block matmuls accumulate with start=False).
            # po[p, b, 2t+j, d] = out[base + 512b + 4p + 2t + j, d]
            po = ps_o.tile([128, 2, 4, 64], F32, tag="po")
            nc.tensor.matmul(po, ones, b2r, start=True, stop=False)

            # L2 feature maps; maps keep h's index order [d1, j, b, t, p].
            # Cross the j-banks between ACT and DVE so the two engines never
            # contend on the same PSUM bank; c2 runs on GpSimd (SBUF only).
            sl2 = f2.tile([128, 2, 2, 2, 128], BF16, tag="sl2")
            u2 = f2.tile([128, 2, 2, 2, 128], BF16, tag="u2")
            c2 = f2.tile([128, 2, 2, 2, 128], BF16, tag="c2")
            nc.scalar.activation(sl2[:, 0], h[:, 0], SILU, bias=b1)
            nc.vector.tensor_scalar(u2[:, 1], h[:, 1], s1, s2, op0=MAX, op1=MIN)
            nc.scalar.activation(sl2[:, 1], h[:, 1], SILU, bias=b1)
            nc.vector.tensor_scalar(u2[:, 0], h[:, 0], s1, s2, op0=MAX, op1=MIN)
            nc.gpsimd.tensor_scalar_max(c2, u2, nb1)

            # chunk-major; groups (j, b, t) ordered by which j-half of the map
            # is ready first (sl2 fills j=0 then j=1; u2 fills j=1 then j=0)
            jbt = [(j, b, t) for j in range(2) for b in range(2) for t in range(2)]
            plan = [
                (0, sl2, jbt),
                (1, u2, jbt[4:] + jbt[:4]),
                (2, c2, jbt),
            ]
            for ci, (c, ft2, order) in enumerate(plan):
                for gi, (j, b, t) in enumerate(order):
                    nc.tensor.matmul(
                        po[:, b, 2 * t + j],
                        ft2[:, j, b, t],
                        w2c[c],
                        start=False,
                        stop=(ci == 2 and gi == 7),
                    )

            ot = osb.tile([128, 2, 4, 64], F32, tag="ot")
            if m % 2 == 0:
                nc.scalar.copy(ot, po)
            else:
                nc.vector.tensor_copy(ot, po)
            dst = bass.AP(
                outd, base * 64, [[4 * 64, 128], [512 * 64, 2], [64, 4], [1, 64]]
            )
            nc.sync.dma_start(dst, ot)

    nc.compile()
    return nc


def _get_nc(rows):
    if rows not in _nc_cache:
        _nc_cache[rows] = _build(rows)
    return _nc_cache[rows]


def kernel(x, cp0, bw0, sw0, imp0, cp1, bw1, sw1, imp1, _trace=False, _trace_kwargs=None):
    x = np.ascontiguousarray(np.asarray(x, dtype=np.float32))
    consts = _prep_consts(
        *[np.asarray(a, dtype=np.float32) for a in (cp0, bw0, sw0, imp0, cp1, bw1, sw1, imp1)]
    )
    rows = x.shape[0] // N_CORES
    nc = _get_nc(rows)
    in_maps = []
    for i in range(N_CORES):
        m = dict(consts)
        m["x"] = x[i * rows : (i + 1) * rows]
        in_maps.append(m)
    res = run_bass_kernel_spmd(
        nc, in_maps, list(range(N_CORES)), trace=_trace, **(_trace_kwargs or {})
    )
    out = np.concatenate([res.results[i]["out"] for i in range(N_CORES)], axis=0)
    if _trace:
        return out, res
    return out


# revision 3
# speedup vs baseline: 4.3589x; 4.3589x over previous
"""Fused 2-layer KAN for Trainium2, data-parallel across 8 NeuronCores.

Math: with G=3 grid points the spline basis is piecewise-linear in x, so each
KAN layer collapses to a small dense matmul over 3 cheap feature maps:

    out = bias + silu(x) @ Wb + u @ P1 + C @ (P2 - P1)
      u = clip(x, -1, 1),  C = max(u, 0)
      Wb = imp*bw;  T = imp*sw*cp;  P1 = T@(bv1-bv0);  P2 = T@(bv2-bv1)
      bias_j = sum_i T[i,j,:] @ bv1

All K=5 spline control points fold into P1/P2/bias on the host (O(I*J*K) work).

Device layout (per 1024-row macro-tile, per core):
  partition p of the input tile holds rows {4p, 4p+1, 4p+2, 4p+3} of a
  512-row half-macro -> every DMA descriptor moves 1 KiB contiguous HBM
  (>=512B line-rate threshold), in and out.
  DMA in (SWDGE f32->bf16 cast) -> PE transpose to feature-major
  -> {silu(ACT), clip(DVE), relu-clip(DVE)} -> L1 row-tiled matmul pairs
  into one 2-bank PSUM tile -> L2 maps (ACT/DVE bank-crossed, c2 on GpSimd)
  -> 24 L2 block matmuls (bias via K=1 ones-matmul PSUM init)
  -> PSUM->SBUF copy (alternating ACT/DVE per macro) -> DMA out (HWDGE).
"""

import os
import sys
from contextlib import ExitStack

import numpy as np
import ml_dtypes

for _p in ("/opt/trn_rl_repo",):
    if _p not in sys.path and os.path.isdir(_p):
        sys.path.insert(0, _p)

import concourse.bass as bass
import concourse.tile as tile
from concourse import bacc, mybir
from concourse.bass_utils import run_bass_kernel_spmd
from concourse.masks import make_identity

F32 = mybir.dt.float32
BF16 = mybir.dt.bfloat16
BF = ml_dtypes.bfloat16

N_CORES = 8
D0, D1, D2 = 64, 128, 64
K, DEG, G, LO, HI = 5, 3, 3, -1.0, 1.0
MACRO = 1024  # batch rows per device macro-iteration

_nc_cache = {}


def _basis_table():
    knots = np.linspace(LO - DEG * 0.1, HI + DEG * 0.1, K + DEG + 1)
    grid = np.linspace(LO, HI, G)
    bv = np.zeros((G, K), dtype=np.float32)
    for i in range(K):
        center = (knots[i + DEG // 2] + knots[i + DEG // 2 + 1]) / 2.0
        width = (knots[i + DEG + 1] - knots[i]) / 2.0
        bv[:, i] = np.exp(-(((grid - center) / width) ** 2))
    bv = bv / (bv.sum(axis=1, keepdims=True) + 1e-6)
    return bv


def _prep_consts(cp0, bw0, sw0, imp0, cp1, bw1, sw1, imp1):
    f8 = np.float64
    bv = _basis_table().astype(f8)
    d1, d2 = bv[1] - bv[0], bv[2] - bv[1]

    def fold(cp, bw, sw, imp):
        T = imp.astype(f8)[:, :, None] * sw.astype(f8)[:, :, None] * cp.astype(f8)
        Wb = imp.astype(f8) * bw.astype(f8)
        return Wb, T @ d1, T @ d2, (T @ bv[1]).sum(axis=0)

    Wb0, P10, P20, b1 = fold(cp0, bw0, sw0, imp0)
    Wb1, P11, P21, b2 = fold(cp1, bw1, sw1, imp1)
    bias2_eff = b2 + b1 @ P21

    w1 = np.stack([Wb0, P10, P20 - P10], axis=0)  # [3, 64, 128] lhsT chunks
    w1 = np.concatenate([w1, w1], axis=1)  # duplicate rows for partitions 64-127
    w1 = np.ascontiguousarray(w1.transpose(1, 0, 2)).reshape(128, 384)
    w2 = np.stack([Wb1, P11, P21 - P11], axis=0)  # [3, 128, 64] rhs chunks
    w2 = np.ascontiguousarray(w2.transpose(1, 0, 2)).reshape(128, 192)

    return {
        "wpk": np.concatenate([w1, w2], axis=1).astype(BF),  # [128, 576]
        "spk": np.stack(
            [b1, -1.0 - b1, 1.0 - b1, -b1], axis=1
        ).astype(np.float32),  # [128, 4] = b1|s1|s2|nb1
        "b2row": np.tile(bias2_eff, 8).astype(BF).reshape(1, 512),
    }


def _build(rows):
    assert rows % MACRO == 0
    nc = bacc.Bacc(
        "TRN2",
        target_bir_lowering=False,
        debug=False,
        enable_asserts=False,
        num_devices=N_CORES,
    )
    xd = nc.dram_tensor("x", [rows, D0], F32, kind="ExternalInput")
    wpkd = nc.dram_tensor("wpk", [128, 576], BF16, kind="ExternalInput")
    spkd = nc.dram_tensor("spk", [128, 4], F32, kind="ExternalInput")
    b2d = nc.dram_tensor("b2row", [1, 512], BF16, kind="ExternalInput")
    outd = nc.dram_tensor("out", [rows, D2], F32, kind="ExternalOutput")

    n_macro = rows // MACRO
    MAX, MIN = mybir.AluOpType.max, mybir.AluOpType.min
    SILU = mybir.ActivationFunctionType.Silu

    with tile.TileContext(nc) as tc, ExitStack() as ctx:
        consts = ctx.enter_context(tc.tile_pool(name="consts", bufs=1))
        xin = ctx.enter_context(tc.tile_pool(name="xin", bufs=4))
        f1 = ctx.enter_context(tc.tile_pool(name="f1", bufs=2))
        f2 = ctx.enter_context(tc.tile_pool(name="f2", bufs=2))
        osb = ctx.enter_context(tc.tile_pool(name="osb", bufs=3))
        ps_x = ctx.enter_context(tc.tile_pool(name="ps_x", bufs=2, space="PSUM"))
        ps_h = ctx.enter_context(tc.tile_pool(name="ps_h", bufs=2, space="PSUM"))
        ps_o = ctx.enter_context(tc.tile_pool(name="ps_o", bufs=2, space="PSUM"))

        ident = consts.tile([128, 128], BF16)
        make_identity(nc, ident)
        ones = consts.tile([1, 128], BF16)
        nc.vector.memset(ones, 1.0)
        wpk = consts.tile([128, 576], BF16)
        nc.sync.dma_start(wpk, wpkd.ap())
        spk = consts.tile([128, 4], F32)
        nc.sync.dma_start(spk, spkd.ap())
        b2r = consts.tile([1, 512], BF16)
        nc.sync.dma_start(b2r, b2d.ap())
        b1, s1, s2, nb1 = (spk[:, i : i + 1] for i in range(4))
        w1c = [wpk[:, c * 128 : (c + 1) * 128] for c in range(3)]
        w2c = [wpk[:, 384 + c * 64 : 384 + (c + 1) * 64] for c in range(3)]

        # PE pre-warm: dummy matmuls spanning ~4us while the first DMAs land,
        # so the HAM clock gate opens (1.2 -> 2.4 GHz) before real work.
        warm = ps_o.tile([128, 2, 4, 64], F32, tag="po")
        for _ in range(48):
            nc.tensor.matmul(warm[:, 0, 0, 0:2], ident, ident[:, 0:2], start=True, stop=True)

        for m in range(n_macro):
            base = m * MACRO
            # xt[p, b, t, j, f] = x[base + 512b + 4p + 2t + j, f], bf16 cast.
            # Per partition: 2 chunks (b) of 256 contiguous f32 = 1KiB HBM.
            xt = xin.tile([128, 2, 2, 2, 64], BF16, tag="xt")
            src = bass.AP(
                xd,
                base * 64,
                [[4 * 64, 128], [512 * 64, 2], [2 * 64, 2], [64, 2], [1, 64]],
            )
            nc.gpsimd.dma_start(xt, src)

            # transpose: px[j*64+f, b, t, p] = x[base + 512b + 4p + 2t + j, f]
            # partitions 0-63 = feats of even rows (j=0), 64-127 = odd (j=1)
            px = ps_x.tile([128, 2, 2, 128], BF16, tag="px")
            for b in range(2):
                for t in range(2):
                    nc.tensor.transpose(px[:, b, t], xt[:, b, t], ident)

            # u1 first: then c1 (DVE, from SBUF) overlaps sl1 (ACT, from PSUM)
            u1 = f1.tile([128, 2, 2, 128], BF16, tag="u1")
            nc.vector.tensor_scalar(u1, px, -1.0, 1.0, op0=MAX, op1=MIN)
            sl1 = f1.tile([128, 2, 2, 128], BF16, tag="sl1")
            nc.scalar.activation(sl1, px, SILU)
            c1 = f1.tile([128, 2, 2, 128], BF16, tag="c1")
            nc.vector.tensor_scalar_max(c1, u1, 0.0)

            # L1: two concurrent 64-contraction row-tiled streams into one
            # 2-bank PSUM tile: h[d1, j, b, t, p]; j=0 -> bank A, j=1 -> bank B
            h = ps_h.tile([128, 2, 2, 2, 128], F32, tag="h")
            # chunk order = feature readiness order (u1 -> sl1 -> c1)
            for i, (c, ft) in enumerate([(1, u1), (0, sl1), (2, c1)]):
                nc.tensor.matmul(
                    h[:, 0], w1c[c][0:64], ft[0:64], start=(i == 0), stop=(i == 2)
                )
                nc.tensor.matmul(
                    h[:, 1], w1c[c][64:128], ft[64:128], start=(i == 0), stop=(i == 2)
                )

            # L2 bias init via K=1 ones-matmul (sets has_written on the whole
            # bank so the 24 block matmuls accumulate with start=False).
            # po[p, b, 2t+j, d] = out[base + 512b + 4p + 2t + j, d]
            po = ps_o.tile([128, 2, 4, 64], F32, tag="po")
            nc.tensor.matmul(po, ones, b2r, start=True, stop=False)

            # L2 feature maps; maps keep h's index order [d1, j, b, t, p].
            # Cross the j-banks between ACT and DVE so the two engines never
            # contend on the same PSUM bank; c2 runs on GpSimd (SBUF only).
            sl2 = f2.tile([128, 2, 2, 2, 128], BF16, tag="sl2")
            u2 = f2.tile([128, 2, 2, 2, 128], BF16, tag="u2")
            c2 = f2.tile([128, 2, 2, 2, 128], BF16, tag="c2")
            nc.scalar.activation(sl2[:, 0], h[:, 0], SILU, bias=b1)
            nc.vector.tensor_scalar(u2[:, 1], h[:, 1], s1, s2, op0=MAX, op1=MIN)
            nc.scalar.activation(sl2[:, 1], h[:, 1], SILU, bias=b1)
            nc.vector.tensor_scalar(u2[:, 0], h[:, 0], s1, s2, op0=MAX, op1=MIN)
            nc.vector.tensor_scalar_max(c2, u2, nb1)

            # chunk-major; groups (j, b, t) ordered by which j-half of the map
            # is ready first (sl2 fills j=0 then j=1; u2 fills j=1 then j=0)
            jbt = [(j, b, t) for j in range(2) for b in range(2) for t in range(2)]
            plan = [
                (0, sl2, jbt),
                (1, u2, jbt[4:] + jbt[:4]),
                (2, c2, jbt),
            ]
            for ci, (c, ft2, order) in enumerate(plan):
                for gi, (j, b, t) in enumerate(order):
                    nc.tensor.matmul(
                        po[:, b, 2 * t + j],
                        ft2[:, j, b, t],
                        w2c[c],
                        start=False,
                        stop=(ci == 2 and gi == 7),
                    )

            ot = osb.tile([128, 2, 4, 64], F32, tag="ot")
            if m % 2 == 0:
                nc.scalar.copy(ot, po)
            else:
                nc.vector.tensor_copy(ot, po)
            dst = bass.AP(
                outd, base * 64, [[4 * 64, 128], [512 * 64, 2], [64, 4], [1, 64]]
            )
            nc.sync.dma_start(dst, ot)

    nc.compile()
    return nc


def _get_nc(rows):
    if rows not in _nc_cache:
        _nc_cache[rows] = _build(rows)
    return _nc_cache[rows]


def kernel(x, cp0, bw0, sw0, imp0, cp1, bw1, sw1, imp1, _trace=False, _trace_kwargs=None):
    x = np.ascontiguousarray(np.asarray(x, dtype=np.float32))
    consts = _prep_consts(
        *[np.asarray(a, dtype=np.float32) for a in (cp0, bw0, sw0, imp0, cp1, bw1, sw1, imp1)]
    )
    rows = x.shape[0] // N_CORES
    nc = _get_nc(rows)
    in_maps = []
    for i in range(N_CORES):
        m = dict(consts)
        m["x"] = x[i * rows : (i + 1) * rows]
        in_maps.append(m)
    res = run_bass_kernel_spmd(
        nc, in_maps, list(range(N_CORES)), trace=_trace, **(_trace_kwargs or {})
    )
    out = np.concatenate([res.results[i]["out"] for i in range(N_CORES)], axis=0)
    if _trace:
        return out, res
    return out
